# revision 28
# baseline (speedup 1.0000x reference)
"""Distributed Trainium2 Bass kernel for causal multi-head attention with RoPE.

Problem: B=2, T=2048, C=1024, H=16 heads, D=64. 8 NeuronCores.

Sharding (2x4 grid): core c handles batch b = c//4 and the 4 heads
g = c%4 -> heads [4g..4g+4). QKV projections + RoPE + causal attention run
fully locally per core in a "transposed" layout (qT/kT = [D_heads, T]) so
no on-chip transposes are ever needed:

  qT = Wq_slice.T @ x.T            (lhsT = Wq natural, rhs = x.T)
  scoresT[k,q] = kT.T-block @ qT   (softmax along PARTITION axis)
  outT = [v|1].T @ exp(scoresT)    (ones column yields softmax denominators)
  outW = Wo_cols.T @ attn_allT     (attn stays transposed through Wo)

The attention output (pre-Wo, [256, T] bf16 per core) is AllGathered inside
each 4-core group, then each core computes its 256-row slice of the
(transposed) Wo projection for the full T. Host-side only reshapes/shards:
x transpose, weight slicing, bf16 casts, cos/sin table layout. All matmuls
run in bf16 (fp32 PSUM accumulation); softmax runs in fp32.
"""

import numpy as np
import ml_dtypes

import concourse.bacc as bacc
import concourse.mybir as mybir
import concourse.tile as tile
from concourse.bass_utils import run_bass_kernel_spmd

B, T, C, H, D = 2, 2048, 1024, 16, 64
NCORES = 8
HPC = 4              # heads per core
CPC = HPC * D        # channels per core (256)
NPAIR = 2            # head pairs per core
QC = 4               # q-chunks of 512
KB = T // 128        # k-blocks of 128
CCH = C // 128       # contraction chunks of 128
F32 = mybir.dt.float32
BF16 = mybir.dt.bfloat16
AF = mybir.ActivationFunctionType
RGROUPS = [[0, 1, 2, 3], [4, 5, 6, 7]]

_cache = {}


def _build_nc():
    nc = bacc.Bacc(None, target_bir_lowering=False, debug=False, num_devices=NCORES)

    xT = nc.declare_dram_parameter("xT", [C, T], BF16, isOutput=False)
    wq = nc.declare_dram_parameter("wq", [C, CPC], BF16, isOutput=False)
    wk = nc.declare_dram_parameter("wk", [C, CPC], BF16, isOutput=False)
    wv = nc.declare_dram_parameter("wv", [C, CPC], BF16, isOutput=False)
    wo = nc.declare_dram_parameter("wo", [C, CPC], BF16, isOutput=False)
    cosP = nc.declare_dram_parameter("cosP", [128, T], F32, isOutput=False)
    sinP = nc.declare_dram_parameter("sinP", [128, T], F32, isOutput=False)
    maskut = nc.declare_dram_parameter("maskut", [128, 256], BF16, isOutput=False)
    smat = nc.declare_dram_parameter("smat", [128, 128], BF16, isOutput=False)
    out = nc.declare_dram_parameter("out", [CPC, T], F32, isOutput=True)

    with tile.TileContext(nc) as tc:
        with (
            tc.tile_pool(name="resident", bufs=1) as rp,
            tc.tile_pool(name="rope", bufs=3) as ropep,
            tc.tile_pool(name="expp", bufs=12) as expp,
            tc.tile_pool(name="outb", bufs=4) as outbp,
            tc.tile_pool(name="agsb", bufs=16) as agp,
            tc.tile_pool(name="small", bufs=4) as smp,
            tc.tile_pool(name="ps", bufs=2, space="PSUM") as psp,
            tc.tile_pool(name="pav", bufs=1, space="PSUM") as pav,
            tc.tile_pool(name="dram", bufs=1, space="DRAM") as dram,
        ):
            # ---------------- resident SBUF ----------------
            xbf = rp.tile([128, CCH * T], BF16)          # x.T in [nch][cc] blocks
            wqbf = rp.tile([128, CCH * CPC], BF16)
            wkbf = rp.tile([128, CCH * CPC], BF16)
            wvbf = rp.tile([128, CCH * CPC], BF16)
            wobf = rp.tile([128, CCH * CPC], BF16)
            cos_sb = rp.tile([128, T], F32)
            sin_sb = rp.tile([128, T], F32)
            mask_bf = rp.tile([128, 256], BF16)
            smat_bf = rp.tile([128, 128], BF16)
            ones_sb = rp.tile([1, 64], BF16)
            qTbf = rp.tile([128, NPAIR * T], BF16)       # rope'd qT, per pair
            kTbf = rp.tile([128, NPAIR * T], BF16)
            vsb = rp.tile([128, HPC * KB * 65], BF16)    # [v | 1] per head per k-block

            # ---------------- load (bf16 direct), spread across DMA queues ----
            nc.sync.dma_start(mask_bf[:], maskut[:])
            nc.sync.dma_start(smat_bf[:], smat[:])
            nc.gpsimd.memset(ones_sb[:], 1.0)

            qeng = [nc.sync, nc.scalar, nc.gpsimd]
            for cc in range(CCH):
                for j, (w_in, w_sb) in enumerate(((wq, wqbf), (wk, wkbf))):
                    qeng[(cc + j) % 3].dma_start(
                        w_sb[:, cc * CPC:(cc + 1) * CPC], w_in[cc * 128:(cc + 1) * 128, :])
            for nch in range(4):
                for cc in range(CCH):
                    qeng[(nch + cc) % 3].dma_start(
                        xbf[:, (nch * CCH + cc) * 512:(nch * CCH + cc + 1) * 512],
                        xT[cc * 128:(cc + 1) * 128, nch * 512:(nch + 1) * 512])
                if nch == 0:
                    nc.gpsimd.dma_start(cos_sb[:], cosP[:])
                    nc.gpsimd.dma_start(sin_sb[:], sinP[:])
                    for cc in range(CCH):
                        for j, (w_in, w_sb) in enumerate(((wv, wvbf), (wo, wobf))):
                            qeng[(cc + j) % 3].dma_start(
                                w_sb[:, cc * CPC:(cc + 1) * CPC], w_in[cc * 128:(cc + 1) * 128, :])

            nc.gpsimd.memset(vsb[:], 1.0)

            # ---------------- phase A: QKV + RoPE (nch-major) ----------------
            for nch in range(T // 512):
                for w_sb, t_sb in ((wqbf, qTbf), (wkbf, kTbf)):
                    for p in range(NPAIR):
                        ps_t = pav.tile([128, 512], F32, tag=f"av0{p}",
                                        name=f"pst{nch}_{0 if w_sb is wqbf else 1}_{p}")
                        for cc in range(CCH):
                            nc.tensor.matmul(
                                ps_t[:],
                                w_sb[:, cc * CPC + p * 128: cc * CPC + (p + 1) * 128],
                                xbf[:, (nch * CCH + cc) * 512:(nch * CCH + cc + 1) * 512],
                                start=(cc == 0), stop=(cc == CCH - 1),
                            )
                        # RoPE: out = ps*cos + (S.T@ps)*|sin|
                        nsl = slice(nch * 512, nch * 512 + 512)
                        qub = ropep.tile([128, 512], BF16, tag="qub")
                        nc.scalar.copy(qub[:], ps_t[:])
                        rot = psp.tile([128, 1024], F32, tag="s")
                        nc.tensor.matmul(rot[:, 0:512], smat_bf[:], qub[:], start=True, stop=True)
                        t1 = ropep.tile([128, 512], F32, tag="t1")
                        nc.vector.tensor_mul(t1[:], ps_t[:], cos_sb[:, nsl])
                        t2 = ropep.tile([128, 512], F32, tag="t2")
                        nc.vector.tensor_mul(t2[:], rot[:, 0:512], sin_sb[:, nsl])
                        nc.vector.tensor_add(t_sb[:, p * T + nch * 512: p * T + nch * 512 + 512], t1[:], t2[:])
                # v natural layout -> vsb [128, h*KB*65 + kb*65 + (0..64)]
                for tch in range(4 * nch, 4 * nch + 4):
                    ps_v = pav.tile([128, 512], F32, tag=f"av1{tch % 2}", name=f"psv{tch}")
                    for cc in range(CCH):
                        nc.tensor.matmul(
                            ps_v[:, 0:CPC],
                            xbf[:, (nch * CCH + cc) * 512 + (tch % 4) * 128:
                                (nch * CCH + cc) * 512 + (tch % 4) * 128 + 128],
                            wvbf[:, cc * CPC:(cc + 1) * CPC],
                            start=(cc == 0), stop=(cc == CCH - 1),
                        )
                    for h in range(HPC):
                        base = h * KB * 65 + tch * 65
                        nc.scalar.copy(vsb[:, base: base + 64], ps_v[:, h * 64:(h + 1) * 64])

            # ---------------- phase B: attention per (qchunk, pair) ----------
            # scoresT for both heads of a pair in one 2-bank psum tile
            # ([:, 0:512] / [:, 512:1024], concurrent via PE row tiling);
            # one exp covers both heads.
            ag_outs = []
            bands = [dram.tile([CPC, 512], BF16, tag=f"agin{i}", name=f"band{i}")
                     for i in range(QC - 1)]
            bands3 = [dram.tile([128, 512], BF16, tag=f"agin3{p}", name=f"band3{p}")
                      for p in range(NPAIR)]

            def make_normalize(qc, p, av, emit_ag):
                # Deferred normalize: the DVE reciprocal chain starts as soon
                # as av is final, but the PE-stream ones-matmul is emitted a
                # few kb into the NEXT pair's loop so the in-order PE queue
                # never waits on DVE.
                def flush():
                    ps_bf = psp.tile([128, 1024], F32, tag="s", name=f"psb{qc}_{p}")
                    ps_b = ps_bf[:, 0:512]
                    for i in range(2):
                        nc.tensor.matmul(ps_b[i * 64:(i + 1) * 64, :], ones_sb[:],
                                         _recs[(qc, p, i)][:],
                                         start=True, stop=True, tile_position=(0, i * 64))
                    bc = smp.tile([128, 512], F32, tag="bcs")
                    nc.scalar.copy(bc[:], ps_b[:])
                    ob = outbp.tile([128, 512], BF16, tag="ob", name=f"ob{qc}_{p}")
                    for i in range(2):
                        nc.vector.tensor_mul(ob[i * 64:(i + 1) * 64, :], av[i][0:64, :],
                                             bc[i * 64:(i + 1) * 64, :])
                    if qc == 3:
                        nc.scalar.dma_start(bands3[p][:], ob[:])
                        ag_out = dram.tile([4 * 128, 512], BF16, tag=f"agout3{p}",
                                           name=f"agout3{p}")
                        nc.gpsimd.collective_compute(
                            "AllGather", mybir.AluOpType.bypass,
                            replica_groups=RGROUPS,
                            ins=[bands3[p].opt()], outs=[ag_out.opt()],
                        )
                        ag_outs.append(ag_out)
                    else:
                        nc.scalar.dma_start(bands[qc][p * 128:(p + 1) * 128, :], ob[:])
                        if emit_ag:
                            ag_out = dram.tile([4 * CPC, 512], BF16, tag=f"agout{qc}",
                                               name=f"agout{qc}")
                            nc.gpsimd.collective_compute(
                                "AllGather", mybir.AluOpType.bypass,
                                replica_groups=RGROUPS,
                                ins=[bands[qc].opt()], outs=[ag_out.opt()],
                            )
                            ag_outs.append(ag_out)
                return flush

            def emit_wo(qc):
                # qc==3: gathered in two pair-halves; C-chunk cc lives in
                # half cc%2 at rows (cc//2)*128. Even chunks (half a) are
                # ready before the second AllGather completes.
                if qc == 3:
                    order = [0, 2, 4, 6, 1, 3, 5, 7]
                    srcs = {cc: (ag_outs[3 + cc % 2], (cc // 2) * 128) for cc in range(CCH)}
                else:
                    order = list(range(CCH))
                    srcs = {cc: (ag_outs[qc], cc * 128) for cc in range(CCH)}
                ag_sb = {}
                for cc in order:
                    src, row = srcs[cc]
                    t = agp.tile([128, 512], BF16, name=f"ag_{qc}_{cc}", tag="ag")
                    nc.sync.dma_start(t[:], src[row:row + 128, :])
                    ag_sb[cc] = t
                for mch in range(2):
                    ps_o = pav.tile([128, 512], F32, tag=f"av{mch}{qc % 2}", name=f"pso{qc}_{mch}")
                    for idx, cc in enumerate(order):
                        nc.tensor.matmul(
                            ps_o[:],
                            wobf[:, cc * CPC + mch * 128: cc * CPC + (mch + 1) * 128],
                            ag_sb[cc][:],
                            start=(idx == 0), stop=(idx == CCH - 1),
                        )
                    osb = outbp.tile([128, 512], F32, tag="osb")
                    nc.scalar.copy(osb[:], ps_o[:])
                    nc.sync.dma_start(out[mch * 128:(mch + 1) * 128, qc * 512:(qc + 1) * 512], osb[:])

            _recs = {}
            pending = None
            for qc in range(QC):
                for p in range(NPAIR):
                    av = [pav.tile([65, 512], F32, tag=f"av{p}{i}", name=f"av{qc}_{p}_{i}")
                          for i in range(2)]
                    kmax = 4 * qc + 4
                    for kb in range(kmax):
                        nqs = max(qc * 512, kb * 128)       # first valid q col
                        noff = nqs - qc * 512
                        n = 512 - noff
                        ps_s = psp.tile([128, 1024], F32, tag="s", name=f"pss{qc}_{p}_{kb}")
                        for i in range(2):                   # head-in-pair
                            hs = slice(i * 64, (i + 1) * 64)
                            nc.tensor.matmul(
                                ps_s[:, i * 512: i * 512 + n],
                                kTbf[hs, p * T + kb * 128: p * T + kb * 128 + 128],
                                qTbf[hs, p * T + nqs: p * T + nqs + n],
                                start=True, stop=True,
                                tile_position=(i * 64, 0),
                            )
                        e = expp.tile([128, 1024], BF16, tag="e", name=f"e{qc}_{p}_{kb}")
                        nc.scalar.activation(e[:], ps_s[:], AF.Exp, scale=0.125)
                        if nqs == kb * 128:                  # diagonal block: causal mask
                            ev = e[:].rearrange("p (b c) -> p b c", b=2)[:, :, 0:128]
                            mv = mask_bf[:].rearrange("p (b c) -> p b c", b=2)
                            nc.vector.tensor_mul(ev, ev, mv)
                        for i in range(2):
                            h = 2 * p + i
                            vbase = h * KB * 65 + kb * 65
                            nc.tensor.matmul(
                                av[i][:, noff:512],
                                vsb[:, vbase: vbase + 65],
                                e[:, i * 512: i * 512 + n],
                                start=(kb == 0), stop=(kb == kmax - 1),
                            )
                        if kb == 2 and pending is not None:
                            pending()
                            pending = None
                    # DVE part of the normalize chain starts immediately
                    for i in range(2):
                        dcp = smp.tile([1, 512], F32, tag=f"dcp{i}", name=f"dcp{qc}_{p}_{i}")
                        nc.vector.tensor_copy(dcp[:], av[i][64:65, :])
                        r = smp.tile([1, 512], BF16, tag=f"rec{i}", name=f"rec{qc}_{p}_{i}")
                        rf = smp.tile([1, 512], F32, tag=f"recf{i}", name=f"recf{qc}_{p}_{i}")
                        nc.vector.reciprocal_approx_fast(rf[:], dcp[:])
                        nc.vector.tensor_copy(r[:], rf[:])
                        _recs[(qc, p, i)] = r
                    if pending is not None:
                        pending()
                    pending = make_normalize(qc, p, av, emit_ag=(p == NPAIR - 1 or qc == 3))

            # ---------------- phase C: Wo per band (transposed output) --------
            # outW[m, t] = sum_c Wo[c, m] * attn_allT[c, t]; lhsT = Wo cols.
            emit_wo(0)
            emit_wo(1)
            emit_wo(2)
            if pending is not None:
                pending()
                pending = None
            emit_wo(3)
    return nc


def _get_nc():
    if "nc" not in _cache:
        nc = _build_nc()
        nc.finalize()
        _cache["nc"] = nc
    return _cache["nc"]


def _host_tables(freqs_cos, freqs_sin):
    cosP = np.empty((128, T), np.float32)
    sinP = np.empty((128, T), np.float32)
    for r in range(128):
        i = (r % 64) // 2
        cosP[r] = freqs_cos[:, i]
        sinP[r] = freqs_sin[:, i]
    maskut = np.tile(np.triu(np.ones((128, 128), np.float32)), (1, 2))
    smat = np.zeros((128, 128), np.float32)
    for i in range(64):
        smat[2 * i + 1, 2 * i] = -1.0   # rot[2i] = -q[2i+1]
        smat[2 * i, 2 * i + 1] = 1.0    # rot[2i+1] = +q[2i]
    return cosP, sinP, maskut, smat


def _install_trace_hooks():
    import sys, types
    try:
        import antenv.axon_hooks  # noqa: F401
        return True
    except ImportError:
        pass
    try:
        from trn_agent_boot.trn_boot import _ntff_profile_via_ctypes
        mod = types.ModuleType("antenv.axon_hooks")
        mod._hook = _ntff_profile_via_ctypes("/opt/axon/libaxon_pjrt.so")
        mod.set_axon_ntff_profile_hook = lambda h: setattr(mod, "_hook", h)
        mod.get_axon_ntff_profile_hook = lambda: mod._hook
        sys.modules["antenv.axon_hooks"] = mod
        import antenv
        antenv.axon_hooks = mod
        import concourse.bass_utils as bu
        bu.upload_artifacts = lambda tmpdir: f"file://{tmpdir}"
        return True
    except Exception:
        return False


def _bf16(a):
    return np.ascontiguousarray(a).astype(ml_dtypes.bfloat16)


def kernel(x, freqs_cos, freqs_sin, Wq, Wk, Wv, Wo, _trace=False):
    x = np.asarray(x, np.float32)
    freqs_cos = np.asarray(freqs_cos, np.float32)
    freqs_sin = np.asarray(freqs_sin, np.float32)
    Wq, Wk, Wv, Wo = (np.asarray(w, np.float32) for w in (Wq, Wk, Wv, Wo))
    cosP, sinP, maskut, smat = _host_tables(freqs_cos, freqs_sin)

    in_maps = []
    for c in range(NCORES):
        b, g = c // 4, c % 4
        sl = slice(g * CPC, (g + 1) * CPC)
        in_maps.append({
            "xT": _bf16(x[b].T),
            "wq": _bf16(Wq[:, sl]),
            "wk": _bf16(Wk[:, sl]),
            "wv": _bf16(Wv[:, sl]),
            "wo": _bf16(Wo[:, sl]),
            "cosP": cosP, "sinP": sinP,
            "maskut": _bf16(maskut), "smat": _bf16(smat),
        })

    nc = _get_nc()
    if _trace:
        _trace = _install_trace_hooks()
    res = run_bass_kernel_spmd(nc, in_maps, core_ids=list(range(NCORES)), trace=_trace)
    _cache["last_res"] = res

    out = np.empty((B, T, C), np.float32)
    for c in range(NCORES):
        b, g = c // 4, c % 4
        out[b][:, g * CPC:(g + 1) * CPC] = res.results[c]["out"].T
    return out


# revision 29
# speedup vs baseline: 1.0170x; 1.0170x over previous
"""Distributed Trainium2 Bass kernel for causal multi-head attention with RoPE.

Problem: B=2, T=2048, C=1024, H=16 heads, D=64. 8 NeuronCores.

Sharding (2x4 grid): core c handles batch b = c//4 and the 4 heads
g = c%4 -> heads [4g..4g+4). QKV projections + RoPE + causal attention run
fully locally per core in a "transposed" layout (qT/kT = [D_heads, T]) so
no on-chip transposes are ever needed:

  qT = Wq_slice.T @ x.T            (lhsT = Wq natural, rhs = x.T)
  scoresT[k,q] = kT.T-block @ qT   (softmax along PARTITION axis)
  outT = [v|1].T @ exp(scoresT)    (ones column yields softmax denominators)
  outW = Wo_cols.T @ attn_allT     (attn stays transposed through Wo)

The attention output (pre-Wo, [256, T] bf16 per core) is AllGathered inside
each 4-core group, then each core computes its 256-row slice of the
(transposed) Wo projection for the full T. Host-side only reshapes/shards:
x transpose, weight slicing, bf16 casts, cos/sin table layout. All matmuls
run in bf16 (fp32 PSUM accumulation); softmax runs in fp32.
"""

import numpy as np
import ml_dtypes

import concourse.bacc as bacc
import concourse.mybir as mybir
import concourse.tile as tile
from concourse.bass_utils import run_bass_kernel_spmd

B, T, C, H, D = 2, 2048, 1024, 16, 64
NCORES = 8
HPC = 4              # heads per core
CPC = HPC * D        # channels per core (256)
NPAIR = 2            # head pairs per core
QC = 4               # q-chunks of 512
KB = T // 128        # k-blocks of 128
CCH = C // 128       # contraction chunks of 128
F32 = mybir.dt.float32
BF16 = mybir.dt.bfloat16
AF = mybir.ActivationFunctionType
RGROUPS = [[0, 1, 2, 3], [4, 5, 6, 7]]

_cache = {}


def _build_nc():
    nc = bacc.Bacc(None, target_bir_lowering=False, debug=False, num_devices=NCORES)

    xT = nc.declare_dram_parameter("xT", [C, T], BF16, isOutput=False)
    wq = nc.declare_dram_parameter("wq", [C, CPC], BF16, isOutput=False)
    wk = nc.declare_dram_parameter("wk", [C, CPC], BF16, isOutput=False)
    wv = nc.declare_dram_parameter("wv", [C, CPC], BF16, isOutput=False)
    wo = nc.declare_dram_parameter("wo", [C, CPC], BF16, isOutput=False)
    cosP = nc.declare_dram_parameter("cosP", [128, T], F32, isOutput=False)
    sinP = nc.declare_dram_parameter("sinP", [128, T], F32, isOutput=False)
    maskut = nc.declare_dram_parameter("maskut", [128, 256], BF16, isOutput=False)
    smat = nc.declare_dram_parameter("smat", [128, 128], BF16, isOutput=False)
    out = nc.declare_dram_parameter("out", [CPC, T], F32, isOutput=True)

    with tile.TileContext(nc) as tc:
        with (
            tc.tile_pool(name="resident", bufs=1) as rp,
            tc.tile_pool(name="rope", bufs=3) as ropep,
            tc.tile_pool(name="expp", bufs=10) as expp,
            tc.tile_pool(name="outb", bufs=4) as outbp,
            tc.tile_pool(name="agsb", bufs=12) as agp,
            tc.tile_pool(name="small", bufs=4) as smp,
            tc.tile_pool(name="ps", bufs=2, space="PSUM") as psp,
            tc.tile_pool(name="pav", bufs=1, space="PSUM") as pav,
            tc.tile_pool(name="dram", bufs=1, space="DRAM") as dram,
        ):
            # ---------------- resident SBUF ----------------
            xbf = rp.tile([128, CCH * T], BF16)          # x.T in [nch][cc] blocks
            wqbf = rp.tile([128, CCH * CPC], BF16)
            wkbf = rp.tile([128, CCH * CPC], BF16)
            wvbf = rp.tile([128, CCH * CPC], BF16)
            wobf = rp.tile([128, CCH * CPC], BF16)
            cos_sb = rp.tile([128, T], F32)
            sin_sb = rp.tile([128, T], F32)
            mask_bf = rp.tile([128, 256], BF16)
            smat_bf = rp.tile([128, 128], BF16)
            ones_sb = rp.tile([1, 64], BF16)
            qTbf = rp.tile([128, NPAIR * T], BF16)       # rope'd qT, per pair
            kTbf = rp.tile([128, NPAIR * T], BF16)
            vsb = rp.tile([128, HPC * KB * 65], BF16)    # [v | 1] per head per k-block

            # ---------------- load (bf16 direct), spread across DMA queues ----
            nc.sync.dma_start(mask_bf[:], maskut[:])
            nc.sync.dma_start(smat_bf[:], smat[:])
            nc.gpsimd.memset(ones_sb[:], 1.0)
            nc.gpsimd.dma_start(cos_sb[:], cosP[:])
            nc.gpsimd.dma_start(sin_sb[:], sinP[:])

            qeng = [nc.sync, nc.scalar, nc.gpsimd]
            for cc in range(CCH):
                for j, (w_in, w_sb) in enumerate(((wq, wqbf), (wk, wkbf))):
                    qeng[(cc + j) % 3].dma_start(
                        w_sb[:, cc * CPC:(cc + 1) * CPC], w_in[cc * 128:(cc + 1) * 128, :])
            for nch in range(4):
                for cc in range(CCH):
                    qeng[(nch + cc) % 3].dma_start(
                        xbf[:, (nch * CCH + cc) * 512:(nch * CCH + cc + 1) * 512],
                        xT[cc * 128:(cc + 1) * 128, nch * 512:(nch + 1) * 512])
                if nch == 0:
                    for cc in range(CCH):
                        for j, (w_in, w_sb) in enumerate(((wv, wvbf), (wo, wobf))):
                            qeng[(cc + j) % 3].dma_start(
                                w_sb[:, cc * CPC:(cc + 1) * CPC], w_in[cc * 128:(cc + 1) * 128, :])

            nc.gpsimd.memset(vsb[:], 1.0)

            # ---------------- phase A: QKV + RoPE (nch-major) ----------------
            for nch in range(T // 512):
                for w_sb, t_sb in ((wqbf, qTbf), (wkbf, kTbf)):
                    for p in range(NPAIR):
                        ps_t = pav.tile([128, 512], F32, tag=f"av0{p}",
                                        name=f"pst{nch}_{0 if w_sb is wqbf else 1}_{p}")
                        for cc in range(CCH):
                            nc.tensor.matmul(
                                ps_t[:],
                                w_sb[:, cc * CPC + p * 128: cc * CPC + (p + 1) * 128],
                                xbf[:, (nch * CCH + cc) * 512:(nch * CCH + cc + 1) * 512],
                                start=(cc == 0), stop=(cc == CCH - 1),
                            )
                        # RoPE: out = ps*cos + (S.T@ps)*|sin|
                        nsl = slice(nch * 512, nch * 512 + 512)
                        qub = ropep.tile([128, 512], BF16, tag="qub")
                        nc.scalar.copy(qub[:], ps_t[:])
                        rot = psp.tile([128, 1024], F32, tag="s")
                        nc.tensor.matmul(rot[:, 0:512], smat_bf[:], qub[:], start=True, stop=True)
                        t1 = ropep.tile([128, 512], F32, tag="t1")
                        nc.vector.tensor_mul(t1[:], ps_t[:], cos_sb[:, nsl])
                        t2 = ropep.tile([128, 512], F32, tag="t2")
                        nc.vector.tensor_mul(t2[:], rot[:, 0:512], sin_sb[:, nsl])
                        nc.vector.tensor_add(t_sb[:, p * T + nch * 512: p * T + nch * 512 + 512], t1[:], t2[:])
                # v natural layout -> vsb [128, h*KB*65 + kb*65 + (0..64)]
                for tch in range(4 * nch, 4 * nch + 4):
                    ps_v = pav.tile([128, 512], F32, tag=f"av1{tch % 2}", name=f"psv{tch}")
                    for cc in range(CCH):
                        nc.tensor.matmul(
                            ps_v[:, 0:CPC],
                            xbf[:, (nch * CCH + cc) * 512 + (tch % 4) * 128:
                                (nch * CCH + cc) * 512 + (tch % 4) * 128 + 128],
                            wvbf[:, cc * CPC:(cc + 1) * CPC],
                            start=(cc == 0), stop=(cc == CCH - 1),
                        )
                    for h in range(HPC):
                        base = h * KB * 65 + tch * 65
                        nc.scalar.copy(vsb[:, base: base + 64], ps_v[:, h * 64:(h + 1) * 64])

            # ---------------- phase B: attention per (qchunk, pair) ----------
            # scoresT for both heads of a pair in one 2-bank psum tile
            # ([:, 0:512] / [:, 512:1024], concurrent via PE row tiling);
            # one exp covers both heads.
            ag_outs = []
            bands = [dram.tile([CPC, 512], BF16, tag=f"agin{i}", name=f"band{i}")
                     for i in range(QC - 1)]
            bands3 = [dram.tile([128, 512], BF16, tag=f"agin3{p}", name=f"band3{p}")
                      for p in range(NPAIR)]

            def make_normalize(qc, p, av, emit_ag):
                # Deferred normalize: the DVE reciprocal chain starts as soon
                # as av is final, but the PE-stream ones-matmul is emitted a
                # few kb into the NEXT pair's loop so the in-order PE queue
                # never waits on DVE.
                def flush():
                    ps_bf = psp.tile([128, 1024], F32, tag="s", name=f"psb{qc}_{p}")
                    ps_b = ps_bf[:, 0:512]
                    for i in range(2):
                        nc.tensor.matmul(ps_b[i * 64:(i + 1) * 64, :], ones_sb[:],
                                         _recs[(qc, p, i)][:],
                                         start=True, stop=True, tile_position=(0, i * 64))
                    bc = smp.tile([128, 512], F32, tag="bcs")
                    nc.scalar.copy(bc[:], ps_b[:])
                    ob = outbp.tile([128, 512], BF16, tag="ob", name=f"ob{qc}_{p}")
                    for i in range(2):
                        nc.vector.tensor_mul(ob[i * 64:(i + 1) * 64, :], av[i][0:64, :],
                                             bc[i * 64:(i + 1) * 64, :])
                    if qc == 3:
                        nc.scalar.dma_start(bands3[p][:], ob[:])
                        ag_out = dram.tile([4 * 128, 512], BF16, tag=f"agout3{p}",
                                           name=f"agout3{p}")
                        nc.gpsimd.collective_compute(
                            "AllGather", mybir.AluOpType.bypass,
                            replica_groups=RGROUPS,
                            ins=[bands3[p].opt()], outs=[ag_out.opt()],
                        )
                        ag_outs.append(ag_out)
                    else:
                        nc.scalar.dma_start(bands[qc][p * 128:(p + 1) * 128, :], ob[:])
                        if emit_ag:
                            ag_out = dram.tile([4 * CPC, 512], BF16, tag=f"agout{qc}",
                                               name=f"agout{qc}")
                            nc.gpsimd.collective_compute(
                                "AllGather", mybir.AluOpType.bypass,
                                replica_groups=RGROUPS,
                                ins=[bands[qc].opt()], outs=[ag_out.opt()],
                            )
                            ag_outs.append(ag_out)
                return flush

            def emit_wo(qc):
                # qc==3: gathered in two pair-halves; C-chunk cc lives in
                # half cc%2 at rows (cc//2)*128. Even chunks (half a) are
                # ready before the second AllGather completes.
                if qc == 3:
                    order = [0, 2, 4, 6, 1, 3, 5, 7]
                    srcs = {cc: (ag_outs[3 + cc % 2], (cc // 2) * 128) for cc in range(CCH)}
                else:
                    order = list(range(CCH))
                    srcs = {cc: (ag_outs[qc], cc * 128) for cc in range(CCH)}
                ag_sb = {}
                for cc in order:
                    src, row = srcs[cc]
                    t = agp.tile([128, 512], BF16, name=f"ag_{qc}_{cc}", tag="ag")
                    nc.sync.dma_start(t[:], src[row:row + 128, :])
                    ag_sb[cc] = t
                for mch in range(2):
                    ps_o = pav.tile([128, 512], F32, tag=f"av{mch}{qc % 2}", name=f"pso{qc}_{mch}")
                    for idx, cc in enumerate(order):
                        nc.tensor.matmul(
                            ps_o[:],
                            wobf[:, cc * CPC + mch * 128: cc * CPC + (mch + 1) * 128],
                            ag_sb[cc][:],
                            start=(idx == 0), stop=(idx == CCH - 1),
                        )
                    osb = outbp.tile([128, 512], F32, tag="osb")
                    nc.scalar.copy(osb[:], ps_o[:])
                    nc.sync.dma_start(out[mch * 128:(mch + 1) * 128, qc * 512:(qc + 1) * 512], osb[:])

            _recs = {}
            pending = None
            for qc in range(QC):
                for p in range(NPAIR):
                    av = [pav.tile([65, 512], F32, tag=f"av{p}{i}", name=f"av{qc}_{p}_{i}")
                          for i in range(2)]
                    kmax = 4 * qc + 4
                    for kb in range(kmax):
                        nqs = max(qc * 512, kb * 128)       # first valid q col
                        noff = nqs - qc * 512
                        n = 512 - noff
                        ps_s = psp.tile([128, 1024], F32, tag="s", name=f"pss{qc}_{p}_{kb}")
                        for i in range(2):                   # head-in-pair
                            hs = slice(i * 64, (i + 1) * 64)
                            nc.tensor.matmul(
                                ps_s[:, i * 512: i * 512 + n],
                                kTbf[hs, p * T + kb * 128: p * T + kb * 128 + 128],
                                qTbf[hs, p * T + nqs: p * T + nqs + n],
                                start=True, stop=True,
                                tile_position=(i * 64, 0),
                            )
                        e = expp.tile([128, 1024], BF16, tag="e", name=f"e{qc}_{p}_{kb}")
                        nc.scalar.activation(e[:], ps_s[:], AF.Exp, scale=0.125)
                        if nqs == kb * 128:                  # diagonal block: causal mask
                            ev = e[:].rearrange("p (b c) -> p b c", b=2)[:, :, 0:128]
                            mv = mask_bf[:].rearrange("p (b c) -> p b c", b=2)
                            nc.vector.tensor_mul(ev, ev, mv)
                        for i in range(2):
                            h = 2 * p + i
                            vbase = h * KB * 65 + kb * 65
                            nc.tensor.matmul(
                                av[i][:, noff:512],
                                vsb[:, vbase: vbase + 65],
                                e[:, i * 512: i * 512 + n],
                                start=(kb == 0), stop=(kb == kmax - 1),
                            )
                        if kb == 2 and pending is not None:
                            pending()
                            pending = None
                    # DVE part of the normalize chain starts immediately
                    for i in range(2):
                        dcp = smp.tile([1, 512], F32, tag=f"dcp{i}", name=f"dcp{qc}_{p}_{i}")
                        nc.vector.tensor_copy(dcp[:], av[i][64:65, :])
                        r = smp.tile([1, 512], BF16, tag=f"rec{i}", name=f"rec{qc}_{p}_{i}")
                        rf = smp.tile([1, 512], F32, tag=f"recf{i}", name=f"recf{qc}_{p}_{i}")
                        nc.vector.reciprocal_approx_fast(rf[:], dcp[:])
                        nc.vector.tensor_copy(r[:], rf[:])
                        _recs[(qc, p, i)] = r
                    if pending is not None:
                        pending()
                    pending = make_normalize(qc, p, av, emit_ag=(p == NPAIR - 1 or qc == 3))

            # ---------------- phase C: Wo per band (transposed output) --------
            # outW[m, t] = sum_c Wo[c, m] * attn_allT[c, t]; lhsT = Wo cols.
            emit_wo(0)
            emit_wo(1)
            emit_wo(2)
            if pending is not None:
                pending()
                pending = None
            emit_wo(3)
    return nc


def _get_nc():
    if "nc" not in _cache:
        nc = _build_nc()
        nc.finalize()
        _cache["nc"] = nc
    return _cache["nc"]


def _host_tables(freqs_cos, freqs_sin):
    cosP = np.empty((128, T), np.float32)
    sinP = np.empty((128, T), np.float32)
    for r in range(128):
        i = (r % 64) // 2
        cosP[r] = freqs_cos[:, i]
        sinP[r] = freqs_sin[:, i]
    maskut = np.tile(np.triu(np.ones((128, 128), np.float32)), (1, 2))
    smat = np.zeros((128, 128), np.float32)
    for i in range(64):
        smat[2 * i + 1, 2 * i] = -1.0   # rot[2i] = -q[2i+1]
        smat[2 * i, 2 * i + 1] = 1.0    # rot[2i+1] = +q[2i]
    return cosP, sinP, maskut, smat


def _install_trace_hooks():
    import sys, types
    try:
        import antenv.axon_hooks  # noqa: F401
        return True
    except ImportError:
        pass
    try:
        from trn_agent_boot.trn_boot import _ntff_profile_via_ctypes
        mod = types.ModuleType("antenv.axon_hooks")
        mod._hook = _ntff_profile_via_ctypes("/opt/axon/libaxon_pjrt.so")
        mod.set_axon_ntff_profile_hook = lambda h: setattr(mod, "_hook", h)
        mod.get_axon_ntff_profile_hook = lambda: mod._hook
        sys.modules["antenv.axon_hooks"] = mod
        import antenv
        antenv.axon_hooks = mod
        import concourse.bass_utils as bu
        bu.upload_artifacts = lambda tmpdir: f"file://{tmpdir}"
        return True
    except Exception:
        return False


def _bf16(a):
    return np.ascontiguousarray(a).astype(ml_dtypes.bfloat16)


def kernel(x, freqs_cos, freqs_sin, Wq, Wk, Wv, Wo, _trace=False):
    x = np.asarray(x, np.float32)
    freqs_cos = np.asarray(freqs_cos, np.float32)
    freqs_sin = np.asarray(freqs_sin, np.float32)
    Wq, Wk, Wv, Wo = (np.asarray(w, np.float32) for w in (Wq, Wk, Wv, Wo))
    cosP, sinP, maskut, smat = _host_tables(freqs_cos, freqs_sin)

    in_maps = []
    for c in range(NCORES):
        b, g = c // 4, c % 4
        sl = slice(g * CPC, (g + 1) * CPC)
        in_maps.append({
            "xT": _bf16(x[b].T),
            "wq": _bf16(Wq[:, sl]),
            "wk": _bf16(Wk[:, sl]),
            "wv": _bf16(Wv[:, sl]),
            "wo": _bf16(Wo[:, sl]),
            "cosP": cosP, "sinP": sinP,
            "maskut": _bf16(maskut), "smat": _bf16(smat),
        })

    nc = _get_nc()
    if _trace:
        _trace = _install_trace_hooks()
    res = run_bass_kernel_spmd(nc, in_maps, core_ids=list(range(NCORES)), trace=_trace)
    _cache["last_res"] = res

    out = np.empty((B, T, C), np.float32)
    for c in range(NCORES):
        b, g = c // 4, c % 4
        out[b][:, g * CPC:(g + 1) * CPC] = res.results[c]["out"].T
    return out


# revision 30
# speedup vs baseline: 1.0490x; 1.0315x over previous
"""Distributed Trainium2 Bass kernel for causal multi-head attention with RoPE.

Problem: B=2, T=2048, C=1024, H=16 heads, D=64. 8 NeuronCores.

Sharding (2x4 grid): core c handles batch b = c//4 and the 4 heads
g = c%4 -> heads [4g..4g+4). QKV projections + RoPE + causal attention run
fully locally per core in a "transposed" layout (qT/kT = [D_heads, T]) so
no on-chip transposes are ever needed:

  qT = Wq_slice.T @ x.T            (lhsT = Wq natural, rhs = x.T)
  scoresT[k,q] = kT.T-block @ qT   (softmax along PARTITION axis)
  outT = [v|1].T @ exp(scoresT)    (ones column yields softmax denominators)
  outW = Wo_cols.T @ attn_allT     (attn stays transposed through Wo)

The attention output (pre-Wo, [256, T] bf16 per core) is AllGathered inside
each 4-core group, then each core computes its 256-row slice of the
(transposed) Wo projection for the full T. Host-side only reshapes/shards:
x transpose, weight slicing, bf16 casts, cos/sin table layout. All matmuls
run in bf16 (fp32 PSUM accumulation); softmax runs in fp32.
"""

import numpy as np
import ml_dtypes

import concourse.bacc as bacc
import concourse.mybir as mybir
import concourse.tile as tile
from concourse.bass_utils import run_bass_kernel_spmd

B, T, C, H, D = 2, 2048, 1024, 16, 64
NCORES = 8
HPC = 4              # heads per core
CPC = HPC * D        # channels per core (256)
NPAIR = 2            # head pairs per core
QC = 4               # q-chunks of 512
KB = T // 128        # k-blocks of 128
CCH = C // 128       # contraction chunks of 128
F32 = mybir.dt.float32
BF16 = mybir.dt.bfloat16
AF = mybir.ActivationFunctionType
RGROUPS = [[0, 1, 2, 3], [4, 5, 6, 7]]

_cache = {}


def _build_nc():
    nc = bacc.Bacc(None, target_bir_lowering=False, debug=False, num_devices=NCORES)

    xT = nc.declare_dram_parameter("xT", [C, T], BF16, isOutput=False)
    wq = nc.declare_dram_parameter("wq", [C, CPC], BF16, isOutput=False)
    wk = nc.declare_dram_parameter("wk", [C, CPC], BF16, isOutput=False)
    wv = nc.declare_dram_parameter("wv", [C, CPC], BF16, isOutput=False)
    wo = nc.declare_dram_parameter("wo", [C, CPC], BF16, isOutput=False)
    cosP = nc.declare_dram_parameter("cosP", [128, T], F32, isOutput=False)
    sinP = nc.declare_dram_parameter("sinP", [128, T], F32, isOutput=False)
    maskut = nc.declare_dram_parameter("maskut", [128, 256], BF16, isOutput=False)
    smat = nc.declare_dram_parameter("smat", [128, 128], BF16, isOutput=False)
    out = nc.declare_dram_parameter("out", [CPC, T], F32, isOutput=True)

    with tile.TileContext(nc) as tc:
        with (
            tc.tile_pool(name="resident", bufs=1) as rp,
            tc.tile_pool(name="rope", bufs=3) as ropep,
            tc.tile_pool(name="expp", bufs=10) as expp,
            tc.tile_pool(name="outb", bufs=4) as outbp,
            tc.tile_pool(name="agsb", bufs=12) as agp,
            tc.tile_pool(name="small", bufs=4) as smp,
            tc.tile_pool(name="ps", bufs=2, space="PSUM") as psp,
            tc.tile_pool(name="pav", bufs=1, space="PSUM") as pav,
            tc.tile_pool(name="dram", bufs=1, space="DRAM") as dram,
        ):
            # ---------------- resident SBUF ----------------
            xbf = rp.tile([128, CCH * T], BF16)          # x.T in [nch][cc] blocks
            wqbf = rp.tile([128, CCH * CPC], BF16)
            wkbf = rp.tile([128, CCH * CPC], BF16)
            wvbf = rp.tile([128, CCH * CPC], BF16)
            wobf = rp.tile([128, CCH * CPC], BF16)
            cos_sb = rp.tile([128, T], F32)
            sin_sb = rp.tile([128, T], F32)
            mask_bf = rp.tile([128, 256], BF16)
            smat_bf = rp.tile([128, 128], BF16)
            ones_sb = rp.tile([1, 64], BF16)
            qTbf = rp.tile([128, NPAIR * T], BF16)       # rope'd qT, per pair
            kTbf = rp.tile([128, NPAIR * T], BF16)
            vsb = rp.tile([128, HPC * KB * 65], BF16)    # [v | 1] per head per k-block

            # ---------------- load (bf16 direct), spread across DMA queues ----
            nc.sync.dma_start(mask_bf[:], maskut[:])
            nc.sync.dma_start(smat_bf[:], smat[:])
            nc.gpsimd.memset(ones_sb[:], 1.0)
            nc.gpsimd.dma_start(cos_sb[:], cosP[:])
            nc.gpsimd.dma_start(sin_sb[:], sinP[:])

            qeng = [nc.sync, nc.scalar, nc.gpsimd]
            for cc in range(CCH):
                qeng[cc % 3].dma_start(
                    wqbf[:, cc * CPC:(cc + 1) * CPC], wq[cc * 128:(cc + 1) * 128, :])
                qeng[(cc + 1) % 3].dma_start(
                    wkbf[:, cc * CPC:(cc + 1) * CPC], wk[cc * 128:(cc + 1) * 128, :])
                qeng[(cc + 2) % 3].dma_start(
                    xbf[:, cc * 512:(cc + 1) * 512],
                    xT[cc * 128:(cc + 1) * 128, 0:512])
            for nch in range(4):
                for cc in range(CCH):
                    if nch > 0:
                        qeng[(nch + cc) % 3].dma_start(
                            xbf[:, (nch * CCH + cc) * 512:(nch * CCH + cc + 1) * 512],
                            xT[cc * 128:(cc + 1) * 128, nch * 512:(nch + 1) * 512])
                if nch == 0:
                    for cc in range(CCH):
                        for j, (w_in, w_sb) in enumerate(((wv, wvbf), (wo, wobf))):
                            qeng[(cc + j) % 3].dma_start(
                                w_sb[:, cc * CPC:(cc + 1) * CPC], w_in[cc * 128:(cc + 1) * 128, :])

            nc.gpsimd.memset(vsb[:], 1.0)

            # ---------------- phase A: QKV + RoPE (nch-major) ----------------
            for nch in range(T // 512):
                for w_sb, t_sb in ((wqbf, qTbf), (wkbf, kTbf)):
                    for p in range(NPAIR):
                        ps_t = pav.tile([128, 512], F32, tag=f"av0{p}",
                                        name=f"pst{nch}_{0 if w_sb is wqbf else 1}_{p}")
                        for cc in range(CCH):
                            nc.tensor.matmul(
                                ps_t[:],
                                w_sb[:, cc * CPC + p * 128: cc * CPC + (p + 1) * 128],
                                xbf[:, (nch * CCH + cc) * 512:(nch * CCH + cc + 1) * 512],
                                start=(cc == 0), stop=(cc == CCH - 1),
                            )
                        # RoPE: out = ps*cos + (S.T@ps)*|sin|
                        nsl = slice(nch * 512, nch * 512 + 512)
                        qub = ropep.tile([128, 512], BF16, tag="qub")
                        nc.scalar.copy(qub[:], ps_t[:])
                        rot = psp.tile([128, 1024], F32, tag="s")
                        nc.tensor.matmul(rot[:, 0:512], smat_bf[:], qub[:], start=True, stop=True)
                        t1 = ropep.tile([128, 512], F32, tag="t1")
                        nc.vector.tensor_mul(t1[:], ps_t[:], cos_sb[:, nsl])
                        t2 = ropep.tile([128, 512], F32, tag="t2")
                        nc.vector.tensor_mul(t2[:], rot[:, 0:512], sin_sb[:, nsl])
                        nc.vector.tensor_add(t_sb[:, p * T + nch * 512: p * T + nch * 512 + 512], t1[:], t2[:])
                # v natural layout -> vsb [128, h*KB*65 + kb*65 + (0..64)]
                for tch in range(4 * nch, 4 * nch + 4):
                    ps_v = pav.tile([128, 512], F32, tag=f"av1{tch % 2}", name=f"psv{tch}")
                    for cc in range(CCH):
                        nc.tensor.matmul(
                            ps_v[:, 0:CPC],
                            xbf[:, (nch * CCH + cc) * 512 + (tch % 4) * 128:
                                (nch * CCH + cc) * 512 + (tch % 4) * 128 + 128],
                            wvbf[:, cc * CPC:(cc + 1) * CPC],
                            start=(cc == 0), stop=(cc == CCH - 1),
                        )
                    for h in range(HPC):
                        base = h * KB * 65 + tch * 65
                        nc.scalar.copy(vsb[:, base: base + 64], ps_v[:, h * 64:(h + 1) * 64])

            # ---------------- phase B: attention per (qchunk, pair) ----------
            # scoresT for both heads of a pair in one 2-bank psum tile
            # ([:, 0:512] / [:, 512:1024], concurrent via PE row tiling);
            # one exp covers both heads.
            ag_outs = []
            bands = [dram.tile([CPC, 512], BF16, tag=f"agin{i}", name=f"band{i}")
                     for i in range(QC - 1)]
            bands3 = [dram.tile([128, 512], BF16, tag=f"agin3{p}", name=f"band3{p}")
                      for p in range(NPAIR)]

            def make_normalize(qc, p, av, emit_ag):
                # Deferred normalize: the DVE reciprocal chain starts as soon
                # as av is final, but the PE-stream ones-matmul is emitted a
                # few kb into the NEXT pair's loop so the in-order PE queue
                # never waits on DVE.
                def flush():
                    ps_bf = psp.tile([128, 1024], F32, tag="s", name=f"psb{qc}_{p}")
                    ps_b = ps_bf[:, 0:512]
                    for i in range(2):
                        nc.tensor.matmul(ps_b[i * 64:(i + 1) * 64, :], ones_sb[:],
                                         _recs[(qc, p, i)][:],
                                         start=True, stop=True, tile_position=(0, i * 64))
                    bc = smp.tile([128, 512], F32, tag="bcs")
                    nc.vector.tensor_copy(bc[:], ps_b[:])
                    ob = outbp.tile([128, 512], BF16, tag="ob", name=f"ob{qc}_{p}")
                    for i in range(2):
                        nc.vector.tensor_mul(ob[i * 64:(i + 1) * 64, :], av[i][0:64, :],
                                             bc[i * 64:(i + 1) * 64, :])
                    if qc == 3:
                        nc.scalar.dma_start(bands3[p][:], ob[:])
                        ag_out = dram.tile([4 * 128, 512], BF16, tag=f"agout3{p}",
                                           name=f"agout3{p}")
                        nc.gpsimd.collective_compute(
                            "AllGather", mybir.AluOpType.bypass,
                            replica_groups=RGROUPS,
                            ins=[bands3[p].opt()], outs=[ag_out.opt()],
                        )
                        ag_outs.append(ag_out)
                    else:
                        nc.scalar.dma_start(bands[qc][p * 128:(p + 1) * 128, :], ob[:])
                        if emit_ag:
                            ag_out = dram.tile([4 * CPC, 512], BF16, tag=f"agout{qc}",
                                               name=f"agout{qc}")
                            nc.gpsimd.collective_compute(
                                "AllGather", mybir.AluOpType.bypass,
                                replica_groups=RGROUPS,
                                ins=[bands[qc].opt()], outs=[ag_out.opt()],
                            )
                            ag_outs.append(ag_out)
                return flush

            def emit_wo(qc):
                # qc==3: gathered in two pair-halves; C-chunk cc lives in
                # half cc%2 at rows (cc//2)*128. Even chunks (half a) are
                # ready before the second AllGather completes.
                if qc == 3:
                    order = [0, 2, 4, 6, 1, 3, 5, 7]
                    srcs = {cc: (ag_outs[3 + cc % 2], (cc // 2) * 128) for cc in range(CCH)}
                else:
                    order = list(range(CCH))
                    srcs = {cc: (ag_outs[qc], cc * 128) for cc in range(CCH)}
                ag_sb = {}
                for cc in order:
                    src, row = srcs[cc]
                    t = agp.tile([128, 512], BF16, name=f"ag_{qc}_{cc}", tag="ag")
                    nc.sync.dma_start(t[:], src[row:row + 128, :])
                    ag_sb[cc] = t
                for mch in range(2):
                    ps_o = pav.tile([128, 512], F32, tag=f"av{mch}{qc % 2}", name=f"pso{qc}_{mch}")
                    for idx, cc in enumerate(order):
                        nc.tensor.matmul(
                            ps_o[:],
                            wobf[:, cc * CPC + mch * 128: cc * CPC + (mch + 1) * 128],
                            ag_sb[cc][:],
                            start=(idx == 0), stop=(idx == CCH - 1),
                        )
                    osb = outbp.tile([128, 512], F32, tag="osb")
                    nc.scalar.copy(osb[:], ps_o[:])
                    nc.sync.dma_start(out[mch * 128:(mch + 1) * 128, qc * 512:(qc + 1) * 512], osb[:])

            _recs = {}
            pending = None
            for qc in range(QC):
                for p in range(NPAIR):
                    av = [pav.tile([65, 512], F32, tag=f"av{p}{i}", name=f"av{qc}_{p}_{i}")
                          for i in range(2)]
                    kmax = 4 * qc + 4
                    for kb in range(kmax):
                        nqs = max(qc * 512, kb * 128)       # first valid q col
                        noff = nqs - qc * 512
                        n = 512 - noff
                        ps_s = psp.tile([128, 1024], F32, tag="s", name=f"pss{qc}_{p}_{kb}")
                        for i in range(2):                   # head-in-pair
                            hs = slice(i * 64, (i + 1) * 64)
                            nc.tensor.matmul(
                                ps_s[:, i * 512: i * 512 + n],
                                kTbf[hs, p * T + kb * 128: p * T + kb * 128 + 128],
                                qTbf[hs, p * T + nqs: p * T + nqs + n],
                                start=True, stop=True,
                                tile_position=(i * 64, 0),
                            )
                        e = expp.tile([128, 1024], BF16, tag="e", name=f"e{qc}_{p}_{kb}")
                        nc.scalar.activation(e[:, 0:512 + n], ps_s[:, 0:512 + n], AF.Exp, scale=0.125)
                        if nqs == kb * 128:                  # diagonal block: causal mask
                            ev = e[:].rearrange("p (b c) -> p b c", b=2)[:, :, 0:128]
                            mv = mask_bf[:].rearrange("p (b c) -> p b c", b=2)
                            nc.vector.tensor_mul(ev, ev, mv)
                        for i in range(2):
                            h = 2 * p + i
                            vbase = h * KB * 65 + kb * 65
                            nc.tensor.matmul(
                                av[i][:, noff:512],
                                vsb[:, vbase: vbase + 65],
                                e[:, i * 512: i * 512 + n],
                                start=(kb == 0), stop=(kb == kmax - 1),
                            )
                        if kb == 2 and pending is not None:
                            pending()
                            pending = None
                    # DVE part of the normalize chain starts immediately
                    for i in range(2):
                        dcp = smp.tile([1, 512], F32, tag=f"dcp{i}", name=f"dcp{qc}_{p}_{i}")
                        nc.vector.tensor_copy(dcp[:], av[i][64:65, :])
                        r = smp.tile([1, 512], BF16, tag=f"rec{i}", name=f"rec{qc}_{p}_{i}")
                        rf = smp.tile([1, 512], F32, tag=f"recf{i}", name=f"recf{qc}_{p}_{i}")
                        nc.vector.reciprocal_approx_fast(rf[:], dcp[:])
                        nc.vector.tensor_copy(r[:], rf[:])
                        _recs[(qc, p, i)] = r
                    if pending is not None:
                        pending()
                    pending = make_normalize(qc, p, av, emit_ag=(p == NPAIR - 1 or qc == 3))

            # ---------------- phase C: Wo per band (transposed output) --------
            # outW[m, t] = sum_c Wo[c, m] * attn_allT[c, t]; lhsT = Wo cols.
            if pending is not None:
                pending()
                pending = None
            emit_wo(0)
            emit_wo(1)
            emit_wo(2)
            emit_wo(3)
    return nc


def _get_nc():
    if "nc" not in _cache:
        nc = _build_nc()
        nc.finalize()
        _cache["nc"] = nc
    return _cache["nc"]


def _host_tables(freqs_cos, freqs_sin):
    cosP = np.empty((128, T), np.float32)
    sinP = np.empty((128, T), np.float32)
    for r in range(128):
        i = (r % 64) // 2
        cosP[r] = freqs_cos[:, i]
        sinP[r] = freqs_sin[:, i]
    maskut = np.tile(np.triu(np.ones((128, 128), np.float32)), (1, 2))
    smat = np.zeros((128, 128), np.float32)
    for i in range(64):
        smat[2 * i + 1, 2 * i] = -1.0   # rot[2i] = -q[2i+1]
        smat[2 * i, 2 * i + 1] = 1.0    # rot[2i+1] = +q[2i]
    return cosP, sinP, maskut, smat


def _install_trace_hooks():
    import sys, types
    try:
        import antenv.axon_hooks  # noqa: F401
        return True
    except ImportError:
        pass
    try:
        from trn_agent_boot.trn_boot import _ntff_profile_via_ctypes
        mod = types.ModuleType("antenv.axon_hooks")
        mod._hook = _ntff_profile_via_ctypes("/opt/axon/libaxon_pjrt.so")
        mod.set_axon_ntff_profile_hook = lambda h: setattr(mod, "_hook", h)
        mod.get_axon_ntff_profile_hook = lambda: mod._hook
        sys.modules["antenv.axon_hooks"] = mod
        import antenv
        antenv.axon_hooks = mod
        import concourse.bass_utils as bu
        bu.upload_artifacts = lambda tmpdir: f"file://{tmpdir}"
        return True
    except Exception:
        return False


def _bf16(a):
    return np.ascontiguousarray(a).astype(ml_dtypes.bfloat16)


def kernel(x, freqs_cos, freqs_sin, Wq, Wk, Wv, Wo, _trace=False):
    x = np.asarray(x, np.float32)
    freqs_cos = np.asarray(freqs_cos, np.float32)
    freqs_sin = np.asarray(freqs_sin, np.float32)
    Wq, Wk, Wv, Wo = (np.asarray(w, np.float32) for w in (Wq, Wk, Wv, Wo))
    cosP, sinP, maskut, smat = _host_tables(freqs_cos, freqs_sin)

    in_maps = []
    for c in range(NCORES):
        b, g = c // 4, c % 4
        sl = slice(g * CPC, (g + 1) * CPC)
        in_maps.append({
            "xT": _bf16(x[b].T),
            "wq": _bf16(Wq[:, sl]),
            "wk": _bf16(Wk[:, sl]),
            "wv": _bf16(Wv[:, sl]),
            "wo": _bf16(Wo[:, sl]),
            "cosP": cosP, "sinP": sinP,
            "maskut": _bf16(maskut), "smat": _bf16(smat),
        })

    nc = _get_nc()
    if _trace:
        _trace = _install_trace_hooks()
    res = run_bass_kernel_spmd(nc, in_maps, core_ids=list(range(NCORES)), trace=_trace)
    _cache["last_res"] = res

    out = np.empty((B, T, C), np.float32)
    for c in range(NCORES):
        b, g = c // 4, c % 4
        out[b][:, g * CPC:(g + 1) * CPC] = res.results[c]["out"].T
    return out
